# revision 6
# baseline (speedup 1.0000x reference)
"""ClusterGCN layer on 8 Trainium2 NeuronCores.

Strategy (per sharding hint): shard nodes by cluster so every intra-cluster
edge is device-local.  Per device:
  - stream node tiles: Xl = X @ W on PE, base = dinv^2 * Xl + bias (DVE),
    written straight to the output buffer
  - sparse phase: dma_gather the source rows of the local edges back out of
    the written output, undo the bias, scale by the edge norm, and
    dma_scatter_add into the destination rows.  Edges are packed into
    "rounds" with distinct destinations per round so the SDMA CCE
    read-modify-write never races.
Nodes that appear in no edge ("non-boundary") only need base; boundary
nodes live in a separate output tensor (OUT_B) so the sparse phase only
depends on the first few streamed chunks.
"""

import numpy as np

N = 100000
D = 256
C = 64
M = 8  # cores

_CHUNK = 512  # nodes per streamed chunk (4 PE subtiles of 128)

# Per-call idx caps for the SWDGE gather/scatter ops (multiples of 128).
# Calls always use single_packet=False: coalescing the whole stream into
# one SDMA packet trips the 64-descriptor packet ceiling above ~1024 idxs.
# 2432-idx calls are HW-validated; chunk anything larger.
_GCAP = 2432
_SCAP = 2432


def _build_program(NCAP, B_ROWS, round_sizes, ECAP):
    import concourse.bacc as bacc
    import concourse.mybir as mybir
    from concourse import tile

    f32 = mybir.dt.float32
    i16 = mybir.dt.int16
    Gc = ECAP // 128
    T = NCAP // 128
    n_chunks = NCAP // _CHUNK
    nbc = B_ROWS // _CHUNK  # chunks that land in OUT_B

    nc = bacc.Bacc("TRN2", target_bir_lowering=False, debug=False, num_devices=M)

    XT = nc.dram_tensor("xt", [D, NCAP], f32, kind="ExternalInput")
    W = nc.dram_tensor("w", [D, D], f32, kind="ExternalInput")
    BB = nc.dram_tensor("bias_bc", [128, D], f32, kind="ExternalInput")
    AW = nc.dram_tensor("a_w", [128, T], f32, kind="ExternalInput")
    GI = nc.dram_tensor("gidx", [128, ECAP // 16], i16, kind="ExternalInput")
    SI = nc.dram_tensor("sidx", [128, ECAP // 16], i16, kind="ExternalInput")
    NP = nc.dram_tensor("normp", [128, Gc], f32, kind="ExternalInput")
    OB = nc.dram_tensor("outb", [B_ROWS, D], f32, kind="ExternalOutput")
    OR = nc.dram_tensor("outr", [NCAP - B_ROWS, D], f32, kind="ExternalOutput")

    mult = mybir.AluOpType.mult
    add = mybir.AluOpType.add

    with tile.TileContext(nc) as tc:
        with (
            tc.tile_pool(name="const", bufs=1) as cpool,
            tc.tile_pool(name="xt", bufs=4) as xpool,
            tc.tile_pool(name="base", bufs=4) as bpool,
            tc.tile_pool(name="ps", bufs=3, space="PSUM") as ppool,
            tc.tile_pool(name="gbuf", bufs=1) as gpool,
        ):
            w_sb = cpool.tile([128, 2, D], f32)
            for k in range(2):
                nc.sync.dma_start(w_sb[:, k, :], W[k * 128 : (k + 1) * 128, :])
            bias_sb = cpool.tile([128, D], f32)
            nc.sync.dma_start(bias_sb[:], BB[:])
            a_sb = cpool.tile([128, T], f32)
            nc.sync.dma_start(a_sb[:], AW[:])
            gi_sb = cpool.tile([128, ECAP // 16], i16)
            nc.sync.dma_start(gi_sb[:], GI[:])
            si_sb = cpool.tile([128, ECAP // 16], i16)
            nc.sync.dma_start(si_sb[:], SI[:])
            np_sb = cpool.tile([128, Gc], f32)
            nc.sync.dma_start(np_sb[:], NP[:])

            for c in range(n_chunks):
                xt_t = xpool.tile([128, 2, _CHUNK], f32, tag="xt")
                for k in range(2):
                    nc.sync.dma_start(
                        xt_t[:, k, :],
                        XT[k * 128 : (k + 1) * 128, c * _CHUNK : (c + 1) * _CHUNK],
                    )
                ps = ppool.tile([128, 4, D], f32, tag="ps")
                bt = bpool.tile([128, 4, D], f32, tag="bt")
                for s in range(4):
                    for k in range(2):
                        nc.tensor.matmul(
                            ps[:, s, :],
                            xt_t[:, k, s * 128 : (s + 1) * 128],
                            w_sb[:, k, :],
                            start=(k == 0),
                            stop=(k == 1),
                        )
                    t = c * 4 + s
                    nc.vector.scalar_tensor_tensor(
                        bt[:, s, :], ps[:, s, :], a_sb[:, t : t + 1], bias_sb[:],
                        mult, add,
                    )
                if c < nbc:
                    dview = OB[c * _CHUNK : (c + 1) * _CHUNK, :]
                else:
                    r0 = (c - nbc) * _CHUNK
                    dview = OR[r0 : r0 + _CHUNK, :]
                nc.sync.dma_start(dview.rearrange("(s p) f -> p s f", p=128), bt[:])

            # sparse phase: gather src rows of every edge slot out of OUT_B
            g_sb = gpool.tile([128, Gc, D], f32)
            off = 0
            while off < ECAP:
                n = min(_GCAP, ECAP - off)
                nc.gpsimd.dma_gather(
                    g_sb[:, off // 128 : (off + n) // 128, :],
                    OB[:],
                    gi_sb[:, off // 16 : (off + n) // 16],
                    n,
                    n,
                    D,
                    single_packet=False,
                )
                off += n
            for j in range(Gc):
                nc.vector.tensor_sub(g_sb[:, j, :], g_sb[:, j, :], bias_sb[:])
                nc.vector.tensor_scalar_mul(
                    g_sb[:, j, :], g_sb[:, j, :], np_sb[:, j : j + 1]
                )
            s0 = 0
            for nr in round_sizes:
                off = 0
                while off < nr:
                    n = min(_SCAP, nr - off)
                    a0 = s0 + off
                    nc.gpsimd.dma_scatter_add(
                        OB[:],
                        g_sb[:, a0 // 128 : (a0 + n) // 128, :],
                        si_sb[:, a0 // 16 : (a0 + n) // 16],
                        n,
                        n,
                        D,
                        single_packet=False,
                    )
                    off += n
                s0 += nr

    nc.compile()
    return nc


def _wrap16(arr):
    """slot g -> [g % 16, g // 16], replicated to 128 partitions."""
    w = arr.reshape(-1, 16).T
    return np.tile(w, (8, 1))


def _run_program(nc, in_maps):
    from concourse.bass_utils import run_bass_kernel_spmd

    return run_bass_kernel_spmd(nc, in_maps, core_ids=list(range(M))).results


def _ceil_to(x, m):
    return -(-x // m) * m


def kernel(X, weight, bias, cluster_assignment, edge_index):
    X = np.ascontiguousarray(np.asarray(X, dtype=np.float32))
    weight = np.ascontiguousarray(np.asarray(weight, dtype=np.float32))
    bias = np.asarray(bias, dtype=np.float32)
    cl = np.asarray(cluster_assignment).astype(np.int64)
    ei = np.asarray(edge_index).astype(np.int64)

    src, dst = ei[0], ei[1]
    intra = cl[src] == cl[dst]
    es, ed = src[intra], dst[intra]

    deg = (np.bincount(ed, minlength=N) + 1.0).astype(np.float32)
    dinv = (1.0 / np.sqrt(deg)).astype(np.float32)

    # clusters -> devices, greedy balance by node count
    csize = np.bincount(cl, minlength=C)
    devn = np.zeros(M, dtype=np.int64)
    cdev = np.zeros(C, dtype=np.int64)
    for c in np.argsort(-csize, kind="stable"):
        d = int(np.argmin(devn))
        cdev[c] = d
        devn[d] += csize[c]
    node_dev = cdev[cl]
    edge_dev = node_dev[ed]

    # per-device local layouts
    devs = []
    for d in range(M):
        nodes_d = np.where(node_dev == d)[0]
        em = edge_dev == d
        esd, edd = es[em], ed[em]
        npr = (dinv[edd] / dinv[esd]).astype(np.float32)  # norm' per edge
        B_nodes = np.unique(np.concatenate([esd, edd]))
        notB = np.setdiff1d(nodes_d, B_nodes, assume_unique=True)
        local_order = np.concatenate([B_nodes, notB])
        g2l = np.full(N, -1, dtype=np.int64)
        g2l[local_order] = np.arange(local_order.size)
        ls, ld = g2l[esd], g2l[edd]
        # round assignment: r-th edge of each destination
        o = np.argsort(ld, kind="stable")
        ls, ld, npr = ls[o], ld[o], npr[o]
        starts = np.zeros(ld.size, dtype=np.int64)
        if ld.size:
            runs = np.flatnonzero(np.diff(ld)) + 1
            starts = np.repeat(
                np.concatenate([[0], runs]),
                np.diff(np.concatenate([[0], runs, [ld.size]])),
            )
        rr = np.arange(ld.size) - starts
        devs.append(dict(
            local_order=local_order, nB=B_nodes.size, n_dev=nodes_d.size,
            ls=ls, ld=ld, npr=npr, rr=rr,
        ))

    max_nB = max(dv["nB"] for dv in devs)
    max_nd = max(dv["n_dev"] for dv in devs)
    B_ROWS = _ceil_to(max_nB + 1, _CHUNK)
    NCAP = max(_ceil_to(max_nd, _CHUNK), B_ROWS + _CHUNK)
    R = max((int(dv["rr"].max()) + 1 if dv["rr"].size else 0) for dv in devs)
    R = max(R, 1)
    round_sizes = []
    for r in range(R):
        mx = max(int((dv["rr"] == r).sum()) for dv in devs)
        round_sizes.append(_ceil_to(max(mx, 1), 128))
    ECAP = int(sum(round_sizes))
    PAD_DST = B_ROWS - 1

    # per-device slot arrays
    in_maps = []
    bias_bc = np.ascontiguousarray(np.broadcast_to(bias, (128, D)))
    T = NCAP // 128
    for dv in devs:
        gidx = np.zeros(ECAP, dtype=np.int16)
        sidx = np.full(ECAP, PAD_DST, dtype=np.int16)
        nrm = np.zeros(ECAP, dtype=np.float32)
        s0 = 0
        for r, nr in enumerate(round_sizes):
            m = dv["rr"] == r
            k = int(m.sum())
            gidx[s0 : s0 + k] = dv["ls"][m]
            sidx[s0 : s0 + k] = dv["ld"][m]
            nrm[s0 : s0 + k] = dv["npr"][m]
            s0 += nr
        n_dev = dv["n_dev"]
        lo = dv["local_order"]
        xt = np.zeros((D, NCAP), dtype=np.float32)
        xt[:, :n_dev] = X[lo].T
        a_w = np.zeros(NCAP, dtype=np.float32)
        a_w[:n_dev] = dinv[lo] * dinv[lo]
        in_maps.append({
            "xt": np.ascontiguousarray(xt),
            "w": weight,
            "bias_bc": bias_bc,
            "a_w": np.ascontiguousarray(a_w.reshape(T, 128).T),
            "gidx": np.ascontiguousarray(_wrap16(gidx)),
            "sidx": np.ascontiguousarray(_wrap16(sidx)),
            "normp": np.ascontiguousarray(nrm.reshape(-1, 128).T),
        })

    nc = _build_program(NCAP, B_ROWS, round_sizes, ECAP)
    results = _run_program(nc, in_maps)

    out = np.empty((N, D), dtype=np.float32)
    for d, dv in enumerate(devs):
        rows = np.concatenate([results[d]["outb"], results[d]["outr"]], axis=0)
        out[dv["local_order"]] = rows[: dv["n_dev"]]

    # clusters with no intra edges keep X
    epc = np.bincount(cl[ed], minlength=C)
    inactive = np.where(epc[cl] == 0)[0]
    if inactive.size:
        out[inactive] = X[inactive]
    return out
